# revision 23
# baseline (speedup 1.0000x reference)
"""Trainium2 Bass kernel for nn_Attention_49082886259369.

Computes, per batch b (one batch per NeuronCore, 8 cores data-parallel):
    fac  = tanh(k @ W + q @ U)            [S, D]
    s    = v^T @ fac                      [D, D]
    attn = softmax(s, axis=batch)         <- couples cores: AllReduce max + sum
    out  = v @ attn                       [S, D]

Precision strategy (PE multiplies at ~fp22; fp32 matmul is 4x slower):
  - matmul1 (k@W + q@U) and matmul2 (v^T@fac) use 3-pass bf16 hi/lo split
    (hi*hi + lo*hi + hi*lo), giving ~2^-17 relative product error at 3x
    bf16 cost (still 1.33x faster than native fp32's 4 passes).
    The dim-0 softmax is extremely sensitive to absolute error in s
    (sigma(s) ~ 30, so d(attn)/attn ~ delta_s): single-pass bf16 or f32r
    would give 1-10% output error; the split gives ~5e-4.
  - matmul3 (v @ attn) uses single-pass float32r (full rate, ample accuracy).

Layouts are pre-tiled on the host so every DMA is a clean
[128 partitions x contiguous free] transfer:
  kT/qT/vT: [MT, 128, DT, 128] with element [mi,p,di,sj] = x[mi*128+sj, di*128+p]
  W/U:      [128, DT, D]       with element [p,di,e]     = W[di*128+p, e]
  v:        [MT, 128, D]       with element [mi,p,d]     = v[mi*128+p, d]
"""

import os

import numpy as np
import ml_dtypes

B, S, D = 8, 2048, 1024
NCORES = 8
P = 128
NE = 512  # matmul free-dim tile (one PSUM bank of fp32)

_CACHE: dict = {}


# --------------------------------------------------------------------------
# device kernel builder
# --------------------------------------------------------------------------

def _build_nc(s_dim: int, d_dim: int, n_reps: int = 1, variant: str = "full"):
    import concourse.mybir as mybir
    import concourse.tile as tile
    from concourse import bacc

    F32 = mybir.dt.float32
    F32R = mybir.dt.float32r
    BF16 = mybir.dt.bfloat16
    ACT = mybir.ActivationFunctionType

    MT = s_dim // P          # row tiles of S
    DT = d_dim // P          # row tiles of D
    NH = d_dim // NE         # column halves of D

    nc = bacc.Bacc("TRN2", target_bir_lowering=False, num_devices=NCORES)

    d_kT_hi = nc.dram_tensor("kT_hi", [MT, P, DT, P], BF16, kind="ExternalInput")
    d_kT_lo = nc.dram_tensor("kT_lo", [MT, P, DT, P], BF16, kind="ExternalInput")
    d_qT_hi = nc.dram_tensor("qT_hi", [MT, P, DT, P], BF16, kind="ExternalInput")
    d_qT_lo = nc.dram_tensor("qT_lo", [MT, P, DT, P], BF16, kind="ExternalInput")
    d_W_hi = nc.dram_tensor("W_hi", [P, DT, d_dim], BF16, kind="ExternalInput")
    d_W_lo = nc.dram_tensor("W_lo", [P, DT, d_dim], BF16, kind="ExternalInput")
    d_U_hi = nc.dram_tensor("U_hi", [P, DT, d_dim], BF16, kind="ExternalInput")
    d_U_lo = nc.dram_tensor("U_lo", [P, DT, d_dim], BF16, kind="ExternalInput")
    d_v_hi = nc.dram_tensor("v_hi", [MT, P, d_dim], BF16, kind="ExternalInput")
    d_v_lo = nc.dram_tensor("v_lo", [MT, P, d_dim], BF16, kind="ExternalInput")
    d_vT = nc.dram_tensor("vT", [MT, P, DT, P], F32R, kind="ExternalInput")
    d_out = nc.dram_tensor("out", [s_dim, d_dim], F32, kind="ExternalOutput")

    with tile.TileContext(nc) as tc:
        with (
            tc.tile_pool(name="wu", bufs=1) as wu_pool,
            tc.tile_pool(name="kq", bufs=2) as kq_pool,
            tc.tile_pool(name="vv", bufs=2) as v_pool,
            tc.tile_pool(name="fac", bufs=3) as fac_pool,
            tc.tile_pool(name="spers", bufs=1) as s_pool,
            tc.tile_pool(name="stat", bufs=1) as stat_pool,
            tc.tile_pool(name="vt", bufs=2) as vt_pool,
            tc.tile_pool(name="ost", bufs=2) as out_pool,
            tc.tile_pool(name="fps", bufs=2, space="PSUM") as fac_psum,
            tc.tile_pool(name="sps", bufs=4, space="PSUM") as s_psum,
            tc.tile_pool(name="ops", bufs=2, space="PSUM") as out_psum,
            tc.tile_pool(name="dram", bufs=4, space="DRAM") as dram_pool,
        ):
          for _rep in range(n_reps):
            # s accumulator + attn, one tile PER e-half so half 1's matmuls
            # don't falsely serialize behind half 0's softmax (Tile tracks
            # deps per tile); attn is float32r so matmul3 runs at full rate
            s_half = [
                s_pool.tile([P, DT, NE], F32, tag=f"s_sb{ni}",
                            name=f"s_sb{ni}")
                for ni in range(NH)
            ]
            attn_half = [
                s_pool.tile([P, DT, NE], F32R, tag=f"attn_sb{ni}",
                            name=f"attn_sb{ni}")
                for ni in range(NH)
            ]

            # phase A+B fused per e-half: fac then s; then that half's
            # softmax collectives are issued immediately so they overlap
            # the next half's matmuls (collectives run on TOPSP+SDMA).
            cc_max_tiles = []
            for ni in range(NH):
                esl = slice(ni * NE, (ni + 1) * NE)
                # prefetch mi=0 operands ahead of the bulky W/U loads so the
                # first matmul isn't queued behind 4MB of weight DMA
                pre = {}
                for nm_, dd in (("kth", d_kT_hi), ("ktl", d_kT_lo),
                                ("qth", d_qT_hi), ("qtl", d_qT_lo)):
                    t_ = kq_pool.tile([P, DT, P], BF16, tag=nm_,
                                      name=f"{nm_}p{ni}")
                    nc.sync.dma_start(out=t_, in_=dd[0])
                    pre[nm_] = t_
                for nm_, dd in (("vh", d_v_hi), ("vl", d_v_lo)):
                    t_ = v_pool.tile([P, d_dim], BF16, tag=nm_,
                                     name=f"{nm_}p{ni}")
                    nc.sync.dma_start(out=t_, in_=dd[0])
                    pre[nm_] = t_
                wh, wl, uh, ul = [], [], [], []
                for lst, nm_, dd in (
                    (wh, "wh", d_W_hi), (uh, "uh", d_U_hi),
                    (wl, "wl", d_W_lo), (ul, "ul", d_U_lo),
                ):
                    for di in range(DT):
                        t_ = wu_pool.tile([P, NE], BF16, tag=f"{nm_}{di}",
                                          name=f"{nm_}{di}")
                        nc.sync.dma_start(out=t_, in_=dd[:, di, esl])
                        lst.append(t_)

                for mi in range(MT):
                    if mi == 0:
                        kth, ktl = pre["kth"], pre["ktl"]
                        qth, qtl = pre["qth"], pre["qtl"]
                        vh, vl = pre["vh"], pre["vl"]
                    else:
                        kth = kq_pool.tile([P, DT, P], BF16, tag="kth")
                        ktl = kq_pool.tile([P, DT, P], BF16, tag="ktl")
                        qth = kq_pool.tile([P, DT, P], BF16, tag="qth")
                        qtl = kq_pool.tile([P, DT, P], BF16, tag="qtl")
                        nc.sync.dma_start(out=kth, in_=d_kT_hi[mi])
                        nc.sync.dma_start(out=ktl, in_=d_kT_lo[mi])
                        nc.sync.dma_start(out=qth, in_=d_qT_hi[mi])
                        nc.sync.dma_start(out=qtl, in_=d_qT_lo[mi])
                        vh = v_pool.tile([P, d_dim], BF16, tag="vh")
                        vl = v_pool.tile([P, d_dim], BF16, tag="vl")
                        nc.sync.dma_start(out=vh, in_=d_v_hi[mi])
                        nc.sync.dma_start(out=vl, in_=d_v_lo[mi])

                    # matmul1: fac_pre[mi, esl] = k@W + q@U (3-pass split)
                    fps = fac_psum.tile([P, NE], F32)
                    combos = [
                        (kth, wh), (qth, uh),
                        (ktl, wh), (qtl, uh),
                        (kth, wl), (qth, ul),
                    ]
                    n_acc = len(combos) * DT
                    idx = 0
                    for a_t, b_t in combos:
                        for di in range(DT):
                            nc.tensor.matmul(
                                fps,
                                a_t[:, di, :],
                                b_t[di],
                                start=(idx == 0),
                                stop=(idx == n_acc - 1),
                            )
                            idx += 1

                    stage = fac_pool.tile([P, NE], F32, tag="stage")
                    nc.scalar.activation(stage, fps, ACT.Tanh)
                    fh = fac_pool.tile([P, NE], BF16, tag="fh")
                    fl = fac_pool.tile([P, NE], BF16, tag="fl")
                    nc.vector.tensor_copy(fh, stage)
                    nc.vector.tensor_sub(fl, stage, fh)

                    # matmul2: s[:, esl] += v[mi]^T @ fac[mi, esl] (3-pass)
                    for di in range(DT):
                        sps = s_psum.tile([P, NE], F32)
                        vslh = vh[:, di * P:(di + 1) * P]
                        vsll = vl[:, di * P:(di + 1) * P]
                        nc.tensor.matmul(sps, vslh, fh, start=True, stop=False)
                        nc.tensor.matmul(sps, vsll, fh, start=False, stop=False)
                        nc.tensor.matmul(sps, vslh, fl, start=False, stop=True)
                        dst = s_half[ni][:, di, :]
                        if mi == 0:
                            nc.vector.tensor_copy(dst, sps)
                        else:
                            nc.vector.tensor_add(dst, dst, sps)

                if variant == "nosm":
                    continue
                # issue only the AR-max for this finished half here: the
                # dependent exp/sub work is deferred below so the in-order
                # ACT/DVE streams aren't blocked ahead of the next half's
                # tanh/split work
                s_bf = stat_pool.tile([P, DT, NE], BF16, tag="s_bf")
                nc.vector.tensor_copy(s_bf, s_half[ni])
                cc_s_in = dram_pool.tile([P, DT, NE], BF16, tag="cc_s_in",
                                         name=f"cc_s_in{ni}")
                cc_s_max = dram_pool.tile([P, DT, NE], BF16, tag="cc_s_max",
                                          name=f"cc_s_max{ni}",
                                          addr_space="Shared")
                nc.sync.dma_start(out=cc_s_in, in_=s_bf)
                if variant != "nocc":
                    nc.gpsimd.collective_compute(
                        "AllReduce",
                        mybir.AluOpType.max,
                        replica_groups=[list(range(NCORES))],
                        ins=[cc_s_in.opt()],
                        outs=[cc_s_max.opt()],
                    )
                else:
                    nc.gpsimd.dma_start(out=cc_s_max[:], in_=cc_s_in[:])
                cc_max_tiles.append(cc_s_max)

            if variant == "nosm":
                for mi in range(MT):
                    ost = out_pool.tile([P, d_dim], F32, tag="ost2")
                    nc.vector.tensor_copy(ost[:, :NE], s_half[0][:, 0, :])
                    nc.sync.dma_start(out=d_out[mi * P:(mi + 1) * P, :], in_=ost)
                continue

            # stage 2 per half: exp(s - m), then issue the AR-sum.
            # tile_wait_until + gpsimd DMAs keep these AR-gated ops out of
            # the in-order ACT/DVE/SP streams during the compute sweeps.
            cc_sum_tiles = []
            for ni in range(NH):
                stage2_cm = tc.tile_wait_until(0.40 if ni < NH - 1 else 0.53)
                stage2_cm.__enter__()
                m_sb = stat_pool.tile([P, DT, NE], BF16, tag="m_sb")
                nc.sync.dma_start(out=m_sb, in_=cc_max_tiles[ni])
                sh = s_half[ni]
                for di in range(DT):
                    nc.vector.tensor_sub(
                        sh[:, di, :], sh[:, di, :], m_sb[:, di, :]
                    )
                    nc.scalar.activation(sh[:, di, :], sh[:, di, :], ACT.Exp)
                cc_e_in = dram_pool.tile([P, DT, NE], F32, tag="cc_e_in",
                                         name=f"cc_e_in{ni}")
                cc_e_sum = dram_pool.tile([P, DT, NE], F32, tag="cc_e_sum",
                                          name=f"cc_e_sum{ni}",
                                          addr_space="Shared")
                nc.sync.dma_start(out=cc_e_in, in_=sh)
                if variant != "nocc":
                    nc.gpsimd.collective_compute(
                        "AllReduce",
                        mybir.AluOpType.add,
                        replica_groups=[list(range(NCORES))],
                        ins=[cc_e_in.opt()],
                        outs=[cc_e_sum.opt()],
                    )
                else:
                    nc.gpsimd.dma_start(out=cc_e_sum[:], in_=cc_e_in[:])
                cc_sum_tiles.append(cc_e_sum)
                stage2_cm.__exit__(None, None, None)

            # stage 3 per half: attn = e/den, then matmul3 for that half —
            # half 0's matmuls overlap half 1's AR-sum
            stage3_cm = tc.tile_wait_until(0.54)
            stage3_cm.__enter__()
            for ni in range(NH):
                esl = slice(ni * NE, (ni + 1) * NE)
                den = stat_pool.tile([P, DT, NE], F32, tag="den")
                nc.sync.dma_start(out=den, in_=cc_sum_tiles[ni])
                sh = s_half[ni]
                for di in range(DT):
                    nc.vector.reciprocal(den[:, di, :], den[:, di, :])
                    nc.vector.tensor_mul(
                        attn_half[ni][:, di, :], sh[:, di, :], den[:, di, :]
                    )
                for mi in range(MT):
                    vtt = vt_pool.tile([P, DT, P], F32R, tag="vtt")
                    nc.sync.dma_start(out=vtt, in_=d_vT[mi])
                    ost = out_pool.tile([P, NE], F32, tag="ost")
                    ops = out_psum.tile([P, NE], F32)
                    for di in range(DT):
                        nc.tensor.matmul(
                            ops,
                            vtt[:, di, :],
                            attn_half[ni][:, di, :],
                            start=(di == 0),
                            stop=(di == DT - 1),
                        )
                    nc.vector.tensor_copy(ost, ops)
                    nc.sync.dma_start(
                        out=d_out[mi * P:(mi + 1) * P, esl], in_=ost
                    )
            stage3_cm.__exit__(None, None, None)
            tc.tile_update_base_wait()

    nc.compile()
    return nc


def _get_nc(s_dim=S, d_dim=D, n_reps=1, variant="full"):
    key = ("nc", s_dim, d_dim, n_reps, variant)
    if key not in _CACHE:
        _CACHE[key] = _build_nc(s_dim, d_dim, n_reps, variant)
    return _CACHE[key]


# --------------------------------------------------------------------------
# host-side packing
# --------------------------------------------------------------------------

def _split(x32: np.ndarray):
    """fp32 -> (bf16 hi, bf16 lo) with x ~= hi + lo to ~2^-17 relative."""
    hi = x32.astype(ml_dtypes.bfloat16)
    lo = (x32 - hi.astype(np.float32)).astype(ml_dtypes.bfloat16)
    return hi, lo


def _tileT(x: np.ndarray, s_dim: int, d_dim: int) -> np.ndarray:
    """[S, D] -> [MT, 128, DT, 128] with [mi,p,di,sj] = x[mi*128+sj, di*128+p]."""
    mt, dt = s_dim // P, d_dim // P
    return np.ascontiguousarray(
        x.reshape(mt, P, dt, P).transpose(0, 3, 2, 1)
    )


def prepare_in_maps(q, k, v, W, U, s_dim=S, d_dim=D):
    q = np.asarray(q, dtype=np.float32)
    k = np.asarray(k, dtype=np.float32)
    v = np.asarray(v, dtype=np.float32)
    W = np.asarray(W, dtype=np.float32)
    U = np.asarray(U, dtype=np.float32)

    dt = d_dim // P
    mt = s_dim // P
    W_t = np.ascontiguousarray(W.reshape(dt, P, d_dim).transpose(1, 0, 2))
    U_t = np.ascontiguousarray(U.reshape(dt, P, d_dim).transpose(1, 0, 2))
    W_hi, W_lo = _split(W_t)
    U_hi, U_lo = _split(U_t)

    in_maps = []
    for b in range(NCORES):
        kT = _tileT(k[b], s_dim, d_dim)
        qT = _tileT(q[b], s_dim, d_dim)
        vT = _tileT(v[b], s_dim, d_dim)
        kT_hi, kT_lo = _split(kT)
        qT_hi, qT_lo = _split(qT)
        v_hi, v_lo = _split(v[b].reshape(mt, P, d_dim))
        in_maps.append({
            "kT_hi": kT_hi, "kT_lo": kT_lo,
            "qT_hi": qT_hi, "qT_lo": qT_lo,
            "W_hi": W_hi, "W_lo": W_lo,
            "U_hi": U_hi, "U_lo": U_lo,
            "v_hi": v_hi, "v_lo": v_lo,
            "vT": vT,
        })
    return in_maps


def run_spmd(in_maps, s_dim=S, d_dim=D):
    """One-shot path through the stock bass_utils helper (debug use)."""
    from concourse import bass_utils
    nc = _get_nc(s_dim, d_dim)
    res = bass_utils.run_bass_kernel_spmd(
        nc, in_maps=in_maps, core_ids=list(range(NCORES))
    )
    return res


def _get_runner(s_dim=S, d_dim=D, n_reps=1, variant="full"):
    """Cached sharded-jit runner over the same bass2jax/_bass_exec_p path
    that bass_utils.run_bass_kernel_spmd uses under axon, but built once per
    process (no donation) so repeat calls skip re-trace/re-compile."""
    key = ("runner", s_dim, d_dim, n_reps, variant)
    if key in _CACHE:
        return _CACHE[key]

    import jax
    from jax.sharding import Mesh, PartitionSpec
    from jax.experimental.shard_map import shard_map
    import concourse.mybir as mybir
    from concourse import bass2jax

    nc = _get_nc(s_dim, d_dim, n_reps, variant)
    bass2jax.install_neuronx_cc_hook()

    partition_name = (
        nc.partition_id_tensor.name if nc.partition_id_tensor else None
    )
    in_names, out_names, out_avals, zero_outs = [], [], [], []
    for alloc in nc.m.functions[0].allocations:
        if not isinstance(alloc, mybir.MemoryLocationSet):
            continue
        name = alloc.memorylocations[0].name
        if alloc.kind == "ExternalInput":
            if name != partition_name:
                in_names.append(name)
        elif alloc.kind == "ExternalOutput":
            shape = tuple(alloc.tensor_shape)
            dtype = mybir.dt.np(alloc.dtype)
            out_names.append(name)
            out_avals.append(jax.core.ShapedArray(shape, dtype))
            zero_outs.append(np.zeros(shape, dtype))
    n_params = len(in_names)
    all_in_names = list(in_names) + list(out_names)
    if partition_name is not None:
        all_in_names.append(partition_name)

    def _body(*args):
        operands = list(args)
        if partition_name is not None:
            operands.append(bass2jax.partition_id_tensor())
        outs = bass2jax._bass_exec_p.bind(
            *operands,
            out_avals=tuple(out_avals),
            in_names=tuple(all_in_names),
            out_names=tuple(out_names),
            lowering_input_output_aliases=(),
            sim_require_finite=True,
            sim_require_nnan=True,
            nc=nc,
        )
        return tuple(outs)

    devices = jax.devices()[:NCORES]
    mesh = Mesh(np.asarray(devices), ("core",))
    in_specs = (PartitionSpec("core"),) * (n_params + len(out_names))
    out_specs = (PartitionSpec("core"),) * len(out_names)
    sharded = jax.jit(
        shard_map(
            _body, mesh=mesh, in_specs=in_specs, out_specs=out_specs,
            check_rep=False,
        ),
        keep_unused=True,
    )
    runner = {
        "fn": sharded,
        "in_names": in_names,
        "out_names": out_names,
        "out_avals": out_avals,
        "zero_concat": [
            np.zeros((NCORES * z.shape[0], *z.shape[1:]), z.dtype)
            for z in zero_outs
        ],
        "mesh": mesh,
    }
    _CACHE[key] = runner
    return runner


def _concat_inputs(runner, in_maps):
    return [
        np.concatenate([np.asarray(m[name]) for m in in_maps], axis=0)
        for name in runner["in_names"]
    ]


def run_fast(in_maps, s_dim=S, d_dim=D):
    """Execute via the cached runner; returns list of per-core out dicts."""
    runner = _get_runner(s_dim, d_dim)
    concat_in = _concat_inputs(runner, in_maps)
    out_arrs = runner["fn"](*concat_in, *runner["zero_concat"])
    results = []
    for c in range(NCORES):
        results.append({
            name: np.asarray(out_arrs[i]).reshape(
                NCORES, *runner["out_avals"][i].shape
            )[c]
            for i, name in enumerate(runner["out_names"])
        })
    return results


def timed_run(in_maps, iters=20, s_dim=S, d_dim=D, n_reps=1, variant="full"):
    """Steady-state timing with device-resident inputs. Returns (min_s, all)."""
    import time
    import jax
    from jax.sharding import NamedSharding, PartitionSpec

    runner = _get_runner(s_dim, d_dim, n_reps, variant)
    sh = NamedSharding(runner["mesh"], PartitionSpec("core"))
    dev_in = [jax.device_put(a, sh) for a in _concat_inputs(runner, in_maps)]
    dev_zero = [jax.device_put(z, sh) for z in runner["zero_concat"]]
    jax.block_until_ready(dev_in)
    jax.block_until_ready(dev_zero)
    # warmup (also triggers compile on first use)
    jax.block_until_ready(runner["fn"](*dev_in, *dev_zero))
    times = []
    for _ in range(iters):
        t0 = time.perf_counter()
        jax.block_until_ready(runner["fn"](*dev_in, *dev_zero))
        times.append(time.perf_counter() - t0)
    return min(times), times


def kernel(q, k, v, W, U):
    in_maps = prepare_in_maps(q, k, v, W, U)
    if os.environ.get("BASS_USE_SPMD_HELPER"):
        res = run_spmd(in_maps)
        results = res.results
    else:
        results = run_fast(in_maps)
    out = np.stack([results[b]["out"] for b in range(NCORES)], axis=0)
    return out.astype(np.float32)


def timed_slope(in_maps, iters=30, reps_hi=3, s_dim=S, d_dim=D, variant="full"):
    """True HW kernel time via replication slope: the reps_hi variant runs
    the whole kernel body reps_hi times inside one NEFF. Calls of the two
    variants are interleaved in one loop so slow network drift cancels;
    returns (per_rep_seconds from median pairwise delta, t1_min, thi_min)."""
    import time
    import jax
    from jax.sharding import NamedSharding, PartitionSpec

    runners = {}
    for n in (1, reps_hi):
        r = _get_runner(s_dim, d_dim, n, variant)
        sh = NamedSharding(r["mesh"], PartitionSpec("core"))
        dev_in = [jax.device_put(a, sh) for a in _concat_inputs(r, in_maps)]
        dev_zero = [jax.device_put(z, sh) for z in r["zero_concat"]]
        jax.block_until_ready(dev_in)
        jax.block_until_ready(dev_zero)
        jax.block_until_ready(r["fn"](*dev_in, *dev_zero))  # warm/compile
        runners[n] = (r["fn"], dev_in, dev_zero)

    deltas, t1s, this_ = [], [], []
    for _ in range(iters):
        fn, di, dz = runners[1]
        t0 = time.perf_counter()
        jax.block_until_ready(fn(*di, *dz))
        t1 = time.perf_counter() - t0
        fn, di, dz = runners[reps_hi]
        t0 = time.perf_counter()
        jax.block_until_ready(fn(*di, *dz))
        th = time.perf_counter() - t0
        deltas.append(th - t1)
        t1s.append(t1)
        this_.append(th)
    deltas.sort()
    med = deltas[len(deltas) // 2]
    return med / (reps_hi - 1), min(t1s), min(this_)
